# revision 12
# baseline (speedup 1.0000x reference)
"""Trainium2 Bass kernel for nn_Attention_884763263569.

Per-sample compute: k/v projections per view t, q over the concat, 3-way
softmax attention, small FC head.  Pure data-parallel over 8 NeuronCores.

Layout strategy (per core, NB = B/8 samples):
 - host pre-transposes x to bf16 [ngroups, 128, 12, GROUP] (group-major,
   partition-contiguous) so the PE consumes x chunks directly as stationary
   operands (contraction dim on partitions) with zero on-chip transposes of
   the big tensor, and each DMA descriptor is a contiguous 24KB run.
 - weights fused host-side into wc[12, 128, 96]: for chunk c (t = c//4),
   columns are [Wk.T | Wv.T | Wq_t.T] for that 128-row d-range.
 - per 128-sample subtile: 12 matmuls accumulate y = [128 samples, 288]
   (three 96-wide groups [k_t | v_t | qp_t]) in one PSUM bank.
 - epilogue in sample-major layout: q = sum_t qp_t, logits via mul+reduce,
   exp (no max-subtraction: logits bounded ~±30), u = sum_t e_t*v_t with Z
   appended as column 33, PE-transpose of [128,33], FC matmul with bias
   folded through the Z column, 1/Z scale on ScalarE, natural row-major
   output DMA.
"""

import os
import sys
from contextlib import ExitStack

import numpy as np

sys.path.insert(0, "/opt/trn_rl_repo")

import ml_dtypes

import concourse.bass as bass
import concourse.tile as tile
from concourse import mybir
from concourse.bass_utils import run_bass_kernel_spmd
from concourse.masks import make_identity

BF16 = ml_dtypes.bfloat16

NCORES = 8
T, D, P, C = 3, 512, 32, 10
DF = T * D            # 1536
KC = DF // 128        # 12 d-chunks
GROUP = 1024          # samples per pipeline group
SUB = GROUP // 128    # 128-sample subtiles per group
SLAB = 512            # samples per DMA slab
GPRIO = 150           # ~one group worth of instruction priority
SPG = GROUP // SLAB   # slabs per group


def _ins_dim(ap_obj, pos, size, stride=0):
    """Return a new AP with a [stride, size] dim inserted at position pos."""
    new_ap = [list(d) for d in ap_obj.ap]
    new_ap.insert(pos, [stride, size])
    return bass.AP(tensor=ap_obj.tensor, offset=ap_obj.offset, ap=new_ap)


def build_nc(nb):
    assert nb % GROUP == 0
    ngroups = nb // GROUP

    nc = bass.Bass(target_bir_lowering=False)
    nslabs = nb // SLAB
    xt = nc.declare_dram_parameter(
        "xt", [nslabs, 128, KC, SLAB], mybir.dt.bfloat16, isOutput=False
    )
    wc = nc.declare_dram_parameter("wc", [KC, 128, 96], mybir.dt.bfloat16, isOutput=False)
    wfc = nc.declare_dram_parameter("wfc", [P + 1, C], mybir.dt.float32, isOutput=False)
    out = nc.declare_dram_parameter("out", [nb, C], mybir.dt.float32, isOutput=True)

    f32 = mybir.dt.float32
    bf16 = mybir.dt.bfloat16
    mult = mybir.AluOpType.mult
    add = mybir.AluOpType.add

    with ExitStack() as ctx:
        tc = ctx.enter_context(tile.TileContext(nc))
        wpool = ctx.enter_context(tc.tile_pool(name="wpool", bufs=1))
        xpool = ctx.enter_context(tc.tile_pool(name="xpool", bufs=6))
        ypsum = ctx.enter_context(tc.tile_pool(name="ypsum", bufs=4, space="PSUM"))
        cpsum = ctx.enter_context(tc.tile_pool(name="cpsum", bufs=1, space="PSUM"))
        opsum = ctx.enter_context(tc.tile_pool(name="opsum", bufs=2, space="PSUM"))
        ypool = ctx.enter_context(tc.tile_pool(name="ypool", bufs=3))
        spool = ctx.enter_context(tc.tile_pool(name="spool", bufs=3))
        opool = ctx.enter_context(tc.tile_pool(name="opool", bufs=3))

        # --- persistent tiles ---
        wc_sb = wpool.tile([128, KC, 96], bf16)
        nc.sync.dma_start(out=wc_sb[:], in_=wc.ap().rearrange("c p o -> p c o"))
        wfc_sb = wpool.tile([P + 1, C], f32)
        nc.sync.dma_start(out=wfc_sb[:], in_=wfc.ap())
        ident = wpool.tile([128, 128], f32)
        make_identity(nc, ident[:])

        out_ap = out.ap()
        xt_ap = xt.ap()

        for g in range(ngroups):
            gs = g * GROUP
            # --- load x^T slabs: [128 (d-in-chunk), KC, SLAB] bf16 ---
            slabs = []
            for h in range(SPG):
                xs = xpool.tile([128, KC, SLAB], bf16)
                nc.sync.dma_start(out=xs[:], in_=xt_ap[g * SPG + h])
                slabs.append(xs)

            # --- projections: per 128-sample subtile, 12 matmuls -> y [128, 288]
            y_list = []
            for j in range(SUB):
                y_ps = ypsum.tile([128, 3 * 96], f32)
                xs = slabs[(j * 128) // SLAB]
                jj = (j * 128) % SLAB
                for c in range(KC):
                    t = c // 4
                    nc.tensor.matmul(
                        y_ps[:, 96 * t : 96 * t + 96],
                        xs[:, c, jj : jj + 128],
                        wc_sb[:, c, :],
                        start=(c % 4 == 0),
                        stop=(c % 4 == 3),
                    )
                y_list.append(y_ps)

            # --- copy y PSUM -> SBUF (bf16), split DVE/ACT ---
            Y = ypool.tile([128, SUB, 288], bf16)
            with tc.high_priority(offset=GPRIO):
                for j, y_ps in enumerate(y_list):
                    if j % 2 == 0:
                        nc.vector.tensor_copy(out=Y[:, j, :], in_=y_ps[:, :])
                    else:
                        nc.scalar.copy(out=Y[:, j, :], in_=y_ps[:, :])

            # layout per subtile: [k0 v0 qp0 | k1 v1 qp1 | k2 v2 qp2] blocks of 96
            # (block t at 96*t: k at +0, v at +32, qp at +64)
            # --- q = qp0 + qp1 + qp2 (into the qp0 slot, cols 64:96) ---
            nc.vector.tensor_tensor(
                out=Y[:, :, 64:96], in0=Y[:, :, 64:96], in1=Y[:, :, 160:192], op=add
            )
            nc.vector.tensor_tensor(
                out=Y[:, :, 64:96], in0=Y[:, :, 64:96], in1=Y[:, :, 256:288], op=add
            )

            # --- logits_t = sum_p q*k_t ---
            m_scr = spool.tile([128, SUB, 3, 32], f32)
            q_b = _ins_dim(Y[:, :, 64:96], 2, 3, 0)        # [128, SUB, 3, 32], t bcast
            k_v = _ins_dim(Y[:, :, 0:32], 2, 3, 96)        # k_t at 96*t
            nc.gpsimd.tensor_tensor(out=m_scr[:], in0=q_b, in1=k_v, op=mult)
            logits = spool.tile([128, SUB, 3], f32)
            nc.vector.tensor_reduce(
                out=logits[:], in_=m_scr[:], axis=mybir.AxisListType.X, op=add
            )

            # --- e = exp(logits), Z = sum_t e, R = 1/Z ---
            E = spool.tile([128, SUB, 3], f32)
            nc.scalar.activation(
                out=E[:], in_=logits[:], func=mybir.ActivationFunctionType.Exp
            )
            Z = spool.tile([128, SUB, 1], f32)
            nc.vector.tensor_reduce(
                out=Z[:], in_=E[:], axis=mybir.AxisListType.X, op=add
            )
            R = spool.tile([128, SUB, 1], f32)
            nc.vector.reciprocal(out=R[:], in_=Z[:])

            # --- u = sum_t e_t * v_t ; U33 = [u | Z] ---
            s_scr = spool.tile([128, SUB, 32, 3], f32)
            v_v = _ins_dim(Y[:, :, 32:64], 3, 3, 96)       # dims (g, o, t)
            e_b = _ins_dim(E[:, :, :], 2, 32, 0)           # dims (g, o, t)
            nc.gpsimd.tensor_tensor(out=s_scr[:], in0=v_v, in1=e_b, op=mult)
            U33 = spool.tile([128, SUB, P + 1], f32)
            nc.vector.tensor_reduce(
                out=U33[:, :, 0:32], in_=s_scr[:], axis=mybir.AxisListType.X, op=add
            )
            nc.vector.tensor_copy(out=U33[:, :, 32:33], in_=Z[:])

            # --- cT = U33^T via PE transpose; FC; scale by R ---
            # deferred by ~one group of priority so the PE queue never stalls
            # on the DVE epilogue chain between projection bursts
            ctx_prio = tc.high_priority(offset=-GPRIO)
            ctx_prio.__enter__()
            ct_ps = cpsum.tile([P + 1, GROUP], f32)
            for j in range(SUB):
                nc.tensor.transpose(
                    ct_ps[:, j * 128 : (j + 1) * 128], U33[:, j, :], ident[:]
                )
            ct_sb = spool.tile([P + 1, GROUP], f32)
            nc.scalar.copy(out=ct_sb[:], in_=ct_ps[:])

            o_ps = opsum.tile([128, SUB, C], f32)
            for j in range(SUB):
                nc.tensor.matmul(
                    o_ps[:, j, :],
                    ct_sb[:, j * 128 : (j + 1) * 128],
                    wfc_sb[:],
                    start=True,
                    stop=True,
                )
            out_sb = opool.tile([128, SUB, C], f32)
            r_b = _ins_dim(R[:, :, 0], 2, C, 0)            # [128, SUB, C], bcast over C
            nc.vector.tensor_tensor(out=out_sb[:], in0=o_ps[:], in1=r_b, op=mult)

            nc.scalar.dma_start(
                out=out_ap[gs : gs + GROUP, :].rearrange("(j p) c -> p j c", p=128),
                in_=out_sb[:],
            )
            ctx_prio.__exit__(None, None, None)

    nc.finalize()
    _split_excess_waits(nc)
    return nc


def _split_excess_waits(nc):
    """walrus rejects >1 sync wait on compute instruction structs; hoist the
    extras onto same-engine NoOps inserted just before the offender."""
    exempt = (mybir.InstEventSemaphore,)
    for func in nc.m.functions:
        for blk in func.blocks:
            insts = list(blk.instructions)
            out_list = []
            changed = False
            for inst in insts:
                si = getattr(inst, "sync_info", None)
                ow = list(si.on_wait) if (si is not None and si.on_wait) else []
                if len(ow) > 1 and not isinstance(inst, exempt):
                    for w in ow[:-1]:
                        nop = mybir.InstNoOp(
                            name=nc.get_next_instruction_name(),
                            engine=inst.engine,
                            sync_info=mybir.SyncInfo(on_wait=[w], on_update=[]),
                            bass_nofuse=True,
                        )
                        out_list.append(nop)
                    si.on_wait = [ow[-1]]
                    changed = True
                out_list.append(inst)
            if changed:
                blk.instructions = out_list


_NC_CACHE = {}


def _get_nc(nb):
    if nb not in _NC_CACHE:
        _NC_CACHE[nb] = build_nc(nb)
    return _NC_CACHE[nb]


def _prep_weights(Wk, Wv, Wq, Wfc, bfc):
    WkT = Wk.T.astype(np.float32)   # [512, 32]
    WvT = Wv.T.astype(np.float32)   # [512, 32]
    WqT = Wq.T.astype(np.float32)   # [1536, 32]
    wc = np.zeros((KC, 128, 96), np.float32)
    for c in range(KC):
        t, dsub = divmod(c, 4)
        d512 = slice(dsub * 128, (dsub + 1) * 128)
        rows = slice(c * 128, (c + 1) * 128)
        wc[c, :, 0:32] = WkT[d512]
        wc[c, :, 32:64] = WvT[d512]
        wc[c, :, 64:96] = WqT[rows]
    wc = wc.astype(BF16)
    wfc_aug = np.concatenate(
        [Wfc.T.astype(np.float32), bfc.reshape(1, C).astype(np.float32)], axis=0
    )  # [33, 10]
    return wc, wfc_aug


LAST_RESULT = None


def kernel(x, Wk, Wv, Wq, Wfc, bfc):
    global LAST_RESULT
    x = np.asarray(x, dtype=np.float32)
    Wk = np.asarray(Wk, dtype=np.float32)
    Wv = np.asarray(Wv, dtype=np.float32)
    Wq = np.asarray(Wq, dtype=np.float32)
    Wfc = np.asarray(Wfc, dtype=np.float32)
    bfc = np.asarray(bfc, dtype=np.float32)

    B = x.shape[0]
    assert B % NCORES == 0
    nb = B // NCORES
    nc = _get_nc(nb)
    wc, wfc_aug = _prep_weights(Wk, Wv, Wq, Wfc, bfc)

    xr = x.reshape(NCORES, nb, DF)
    in_maps = []
    for i in range(NCORES):
        # xt[h, p, c, s] = xT[c*128+p, h*SLAB+s]
        xt = np.ascontiguousarray(
            xr[i]
            .astype(BF16)
            .T.reshape(KC, 128, nb // SLAB, SLAB)
            .transpose(2, 1, 0, 3)
        )
        in_maps.append({"xt": xt, "wc": wc, "wfc": wfc_aug})

    LAST_RESULT = run_bass_kernel_spmd(nc, in_maps, core_ids=list(range(NCORES)))
    res = LAST_RESULT.results
    out = np.concatenate([res[i]["out"] for i in range(NCORES)], axis=0)
    return out.astype(np.float32)


# revision 14
# speedup vs baseline: 1.4001x; 1.4001x over previous
"""Trainium2 Bass kernel for nn_Attention_884763263569.

Per-sample compute: k/v projections per view t, q over the concat, 3-way
softmax attention, small FC head.  Pure data-parallel over 8 NeuronCores.

Layout strategy (per core, NB = B/8 samples):
 - host pre-transposes x to bf16 [ngroups, 128, 12, GROUP] (group-major,
   partition-contiguous) so the PE consumes x chunks directly as stationary
   operands (contraction dim on partitions) with zero on-chip transposes of
   the big tensor, and each DMA descriptor is a contiguous 24KB run.
 - weights fused host-side into wc[12, 128, 96]: for chunk c (t = c//4),
   columns are [Wk.T | Wv.T | Wq_t.T] for that 128-row d-range.
 - per 128-sample subtile: 12 matmuls accumulate y = [128 samples, 288]
   (three 96-wide groups [k_t | v_t | qp_t]) in one PSUM bank.
 - epilogue in sample-major layout: q = sum_t qp_t, logits via mul+reduce,
   exp (no max-subtraction: logits bounded ~±30), u = sum_t e_t*v_t with Z
   appended as column 33, PE-transpose of [128,33], FC matmul with bias
   folded through the Z column, 1/Z scale on ScalarE, natural row-major
   output DMA.
"""

import os
import sys
from contextlib import ExitStack

import numpy as np

sys.path.insert(0, "/opt/trn_rl_repo")

import ml_dtypes

import concourse.bass as bass
import concourse.tile as tile
from concourse import mybir
from concourse.bass_utils import run_bass_kernel_spmd
from concourse.masks import make_identity

BF16 = ml_dtypes.bfloat16

NCORES = 8
T, D, P, C = 3, 512, 32, 10
DF = T * D            # 1536
KC = DF // 128        # 12 d-chunks
GROUP = 1024          # samples per pipeline group
SUB = GROUP // 128    # 128-sample subtiles per group
SLAB = 512            # samples per DMA slab
GPRIO = 150           # ~one group worth of instruction priority
SPG = GROUP // SLAB   # slabs per group


def _ins_dim(ap_obj, pos, size, stride=0):
    """Return a new AP with a [stride, size] dim inserted at position pos."""
    new_ap = [list(d) for d in ap_obj.ap]
    new_ap.insert(pos, [stride, size])
    return bass.AP(tensor=ap_obj.tensor, offset=ap_obj.offset, ap=new_ap)


def build_nc(nb):
    assert nb % GROUP == 0
    ngroups = nb // GROUP

    nc = bass.Bass(target_bir_lowering=False)
    nslabs = nb // SLAB
    xt = nc.declare_dram_parameter(
        "xt", [nslabs, 128, KC, SLAB], mybir.dt.bfloat16, isOutput=False
    )
    wc = nc.declare_dram_parameter("wc", [KC, 128, 96], mybir.dt.bfloat16, isOutput=False)
    wfc = nc.declare_dram_parameter("wfc", [P + 1, C], mybir.dt.float32, isOutput=False)
    out = nc.declare_dram_parameter("out", [nb, C], mybir.dt.float32, isOutput=True)

    f32 = mybir.dt.float32
    bf16 = mybir.dt.bfloat16
    mult = mybir.AluOpType.mult
    add = mybir.AluOpType.add

    with ExitStack() as ctx:
        tc = ctx.enter_context(tile.TileContext(nc))
        wpool = ctx.enter_context(tc.tile_pool(name="wpool", bufs=1))
        xpool = ctx.enter_context(tc.tile_pool(name="xpool", bufs=6))
        ypsum = ctx.enter_context(tc.tile_pool(name="ypsum", bufs=4, space="PSUM"))
        cpsum = ctx.enter_context(tc.tile_pool(name="cpsum", bufs=1, space="PSUM"))
        opsum = ctx.enter_context(tc.tile_pool(name="opsum", bufs=2, space="PSUM"))
        ypool = ctx.enter_context(tc.tile_pool(name="ypool", bufs=3))
        spool = ctx.enter_context(tc.tile_pool(name="spool", bufs=3))
        opool = ctx.enter_context(tc.tile_pool(name="opool", bufs=3))

        # --- persistent tiles ---
        wc_sb = wpool.tile([128, KC, 96], bf16)
        nc.sync.dma_start(out=wc_sb[:], in_=wc.ap().rearrange("c p o -> p c o"))
        wfc_sb = wpool.tile([P + 1, C], f32)
        nc.sync.dma_start(out=wfc_sb[:], in_=wfc.ap())
        ident = wpool.tile([128, 128], f32)
        make_identity(nc, ident[:])

        out_ap = out.ap()
        xt_ap = xt.ap()

        for g in range(ngroups):
            gs = g * GROUP
            # --- load x^T slabs: [128 (d-in-chunk), KC, SLAB] bf16 ---
            slabs = []
            for h in range(SPG):
                xs = xpool.tile([128, KC, SLAB], bf16)
                nc.sync.dma_start(out=xs[:], in_=xt_ap[g * SPG + h])
                slabs.append(xs)

            # --- projections: per 128-sample subtile, 12 matmuls -> y [128, 288]
            y_list = []
            for j in range(SUB):
                y_ps = ypsum.tile([128, 3 * 96], f32)
                xs = slabs[(j * 128) // SLAB]
                jj = (j * 128) % SLAB
                for c in range(KC):
                    t = c // 4
                    nc.tensor.matmul(
                        y_ps[:, 96 * t : 96 * t + 96],
                        xs[:, c, jj : jj + 128],
                        wc_sb[:, c, :],
                        start=(c % 4 == 0),
                        stop=(c % 4 == 3),
                    )
                y_list.append(y_ps)

            # --- copy y PSUM -> SBUF (bf16), split DVE/ACT ---
            Y = ypool.tile([128, SUB, 288], bf16)
            with tc.high_priority(offset=GPRIO):
                for j, y_ps in enumerate(y_list):
                    nc.scalar.copy(out=Y[:, j, :], in_=y_ps[:, :])

            # layout per subtile: [k0 v0 qp0 | k1 v1 qp1 | k2 v2 qp2] blocks of 96
            # (block t at 96*t: k at +0, v at +32, qp at +64)
            # --- q = qp0 + qp1 + qp2 (into the qp0 slot, cols 64:96) ---
            nc.vector.tensor_tensor(
                out=Y[:, :, 64:96], in0=Y[:, :, 64:96], in1=Y[:, :, 160:192], op=add
            )
            nc.vector.tensor_tensor(
                out=Y[:, :, 64:96], in0=Y[:, :, 64:96], in1=Y[:, :, 256:288], op=add
            )

            # --- logits_t = sum_p q*k_t ---
            m_scr = spool.tile([128, SUB, 3, 32], f32)
            q_b = _ins_dim(Y[:, :, 64:96], 2, 3, 0)        # [128, SUB, 3, 32], t bcast
            k_v = _ins_dim(Y[:, :, 0:32], 2, 3, 96)        # k_t at 96*t
            nc.vector.tensor_tensor(out=m_scr[:], in0=q_b, in1=k_v, op=mult)
            logits = spool.tile([128, SUB, 3], f32)
            nc.vector.tensor_reduce(
                out=logits[:], in_=m_scr[:], axis=mybir.AxisListType.X, op=add
            )

            # --- e = exp(logits), Z = sum_t e, R = 1/Z ---
            E = spool.tile([128, SUB, 3], f32)
            nc.scalar.activation(
                out=E[:], in_=logits[:], func=mybir.ActivationFunctionType.Exp
            )
            Z = spool.tile([128, SUB, 1], f32)
            nc.vector.tensor_reduce(
                out=Z[:], in_=E[:], axis=mybir.AxisListType.X, op=add
            )
            R = spool.tile([128, SUB, 1], f32)
            nc.vector.reciprocal(out=R[:], in_=Z[:])

            # --- u = sum_t e_t * v_t ; U33 = [u | Z] ---
            s_scr = spool.tile([128, SUB, 32, 3], f32)
            v_v = _ins_dim(Y[:, :, 32:64], 3, 3, 96)       # dims (g, o, t)
            e_b = _ins_dim(E[:, :, :], 2, 32, 0)           # dims (g, o, t)
            nc.vector.tensor_tensor(out=s_scr[:], in0=v_v, in1=e_b, op=mult)
            U33 = spool.tile([128, SUB, P + 1], f32)
            nc.vector.tensor_reduce(
                out=U33[:, :, 0:32], in_=s_scr[:], axis=mybir.AxisListType.X, op=add
            )
            nc.vector.tensor_copy(out=U33[:, :, 32:33], in_=Z[:])

            # --- cT = U33^T via PE transpose; FC; scale by R ---
            # deferred by ~one group of priority so the PE queue never stalls
            # on the DVE epilogue chain between projection bursts
            ctx_prio = tc.high_priority(offset=-GPRIO)
            ctx_prio.__enter__()
            ct_ps = cpsum.tile([P + 1, GROUP], f32)
            for j in range(SUB):
                nc.tensor.transpose(
                    ct_ps[:, j * 128 : (j + 1) * 128], U33[:, j, :], ident[:]
                )
            ct_sb = spool.tile([P + 1, GROUP], f32)
            nc.scalar.copy(out=ct_sb[:], in_=ct_ps[:])

            o_ps = opsum.tile([128, SUB, C], f32)
            for j in range(SUB):
                nc.tensor.matmul(
                    o_ps[:, j, :],
                    ct_sb[:, j * 128 : (j + 1) * 128],
                    wfc_sb[:],
                    start=True,
                    stop=True,
                )
            out_sb = opool.tile([128, SUB, C], f32)
            r_b = _ins_dim(R[:, :, 0], 2, C, 0)            # [128, SUB, C], bcast over C
            nc.vector.tensor_tensor(out=out_sb[:], in0=o_ps[:], in1=r_b, op=mult)

            nc.scalar.dma_start(
                out=out_ap[gs : gs + GROUP, :].rearrange("(j p) c -> p j c", p=128),
                in_=out_sb[:],
            )
            ctx_prio.__exit__(None, None, None)

    nc.finalize()
    _split_excess_waits(nc)
    return nc


def _split_excess_waits(nc):
    """walrus rejects >1 sync wait on compute instruction structs; hoist the
    extras onto same-engine NoOps inserted just before the offender."""
    exempt = (mybir.InstEventSemaphore,)
    for func in nc.m.functions:
        for blk in func.blocks:
            insts = list(blk.instructions)
            out_list = []
            changed = False
            for inst in insts:
                si = getattr(inst, "sync_info", None)
                ow = list(si.on_wait) if (si is not None and si.on_wait) else []
                if len(ow) > 1 and not isinstance(inst, exempt):
                    for w in ow[:-1]:
                        nop = mybir.InstNoOp(
                            name=nc.get_next_instruction_name(),
                            engine=inst.engine,
                            sync_info=mybir.SyncInfo(on_wait=[w], on_update=[]),
                            bass_nofuse=True,
                        )
                        out_list.append(nop)
                    si.on_wait = [ow[-1]]
                    changed = True
                out_list.append(inst)
            if changed:
                blk.instructions = out_list


_NC_CACHE = {}


def _get_nc(nb):
    if nb not in _NC_CACHE:
        _NC_CACHE[nb] = build_nc(nb)
    return _NC_CACHE[nb]


def _prep_weights(Wk, Wv, Wq, Wfc, bfc):
    WkT = Wk.T.astype(np.float32)   # [512, 32]
    WvT = Wv.T.astype(np.float32)   # [512, 32]
    WqT = Wq.T.astype(np.float32)   # [1536, 32]
    wc = np.zeros((KC, 128, 96), np.float32)
    for c in range(KC):
        t, dsub = divmod(c, 4)
        d512 = slice(dsub * 128, (dsub + 1) * 128)
        rows = slice(c * 128, (c + 1) * 128)
        wc[c, :, 0:32] = WkT[d512]
        wc[c, :, 32:64] = WvT[d512]
        wc[c, :, 64:96] = WqT[rows]
    wc = wc.astype(BF16)
    wfc_aug = np.concatenate(
        [Wfc.T.astype(np.float32), bfc.reshape(1, C).astype(np.float32)], axis=0
    )  # [33, 10]
    return wc, wfc_aug


LAST_RESULT = None


def kernel(x, Wk, Wv, Wq, Wfc, bfc):
    global LAST_RESULT
    x = np.asarray(x, dtype=np.float32)
    Wk = np.asarray(Wk, dtype=np.float32)
    Wv = np.asarray(Wv, dtype=np.float32)
    Wq = np.asarray(Wq, dtype=np.float32)
    Wfc = np.asarray(Wfc, dtype=np.float32)
    bfc = np.asarray(bfc, dtype=np.float32)

    B = x.shape[0]
    assert B % NCORES == 0
    nb = B // NCORES
    nc = _get_nc(nb)
    wc, wfc_aug = _prep_weights(Wk, Wv, Wq, Wfc, bfc)

    xr = x.reshape(NCORES, nb, DF)
    in_maps = []
    for i in range(NCORES):
        # xt[h, p, c, s] = xT[c*128+p, h*SLAB+s]
        xt = np.ascontiguousarray(
            xr[i]
            .astype(BF16)
            .T.reshape(KC, 128, nb // SLAB, SLAB)
            .transpose(2, 1, 0, 3)
        )
        in_maps.append({"xt": xt, "wc": wc, "wfc": wfc_aug})

    LAST_RESULT = run_bass_kernel_spmd(nc, in_maps, core_ids=list(range(NCORES)))
    res = LAST_RESULT.results
    out = np.concatenate([res[i]["out"] for i in range(NCORES)], axis=0)
    return out.astype(np.float32)


# revision 16
# speedup vs baseline: 1.4133x; 1.0094x over previous
"""Trainium2 Bass kernel for nn_Attention_884763263569.

Per-sample compute: k/v projections per view t, q over the concat, 3-way
softmax attention, small FC head.  Pure data-parallel over 8 NeuronCores.

Layout strategy (per core, NB = B/8 samples):
 - host pre-transposes x to bf16 [ngroups, 128, 12, GROUP] (group-major,
   partition-contiguous) so the PE consumes x chunks directly as stationary
   operands (contraction dim on partitions) with zero on-chip transposes of
   the big tensor, and each DMA descriptor is a contiguous 24KB run.
 - weights fused host-side into wc[12, 128, 96]: for chunk c (t = c//4),
   columns are [Wk.T | Wv.T | Wq_t.T] for that 128-row d-range.
 - per 128-sample subtile: 12 matmuls accumulate y = [128 samples, 288]
   (three 96-wide groups [k_t | v_t | qp_t]) in one PSUM bank.
 - epilogue in sample-major layout: q = sum_t qp_t, logits via mul+reduce,
   exp (no max-subtraction: logits bounded ~±30), u = sum_t e_t*v_t with Z
   appended as column 33, PE-transpose of [128,33], FC matmul with bias
   folded through the Z column, 1/Z scale on ScalarE, natural row-major
   output DMA.
"""

import os
import sys
from contextlib import ExitStack

import numpy as np

sys.path.insert(0, "/opt/trn_rl_repo")

import ml_dtypes

import concourse.bass as bass
import concourse.tile as tile
from concourse import mybir
from concourse.bass_utils import run_bass_kernel_spmd
from concourse.masks import make_identity

BF16 = ml_dtypes.bfloat16

NCORES = 8
T, D, P, C = 3, 512, 32, 10
DF = T * D            # 1536
KC = DF // 128        # 12 d-chunks
GROUP = 1024          # samples per pipeline group
SUB = GROUP // 128    # 128-sample subtiles per group
SLAB = 512            # samples per DMA slab
GPRIO = 150           # ~one group worth of instruction priority
SPG = GROUP // SLAB   # slabs per group


def _ins_dim(ap_obj, pos, size, stride=0):
    """Return a new AP with a [stride, size] dim inserted at position pos."""
    new_ap = [list(d) for d in ap_obj.ap]
    new_ap.insert(pos, [stride, size])
    return bass.AP(tensor=ap_obj.tensor, offset=ap_obj.offset, ap=new_ap)


def build_nc(nb):
    assert nb % GROUP == 0
    ngroups = nb // GROUP

    nc = bass.Bass(target_bir_lowering=False)
    nslabs = nb // SLAB
    xt = nc.declare_dram_parameter(
        "xt", [nslabs, 128, KC, SLAB], mybir.dt.bfloat16, isOutput=False
    )
    wc = nc.declare_dram_parameter("wc", [KC, 128, 96], mybir.dt.bfloat16, isOutput=False)
    wfc = nc.declare_dram_parameter("wfc", [P + 1, C], mybir.dt.float32, isOutput=False)
    out = nc.declare_dram_parameter("out", [nb, C], mybir.dt.float32, isOutput=True)

    f32 = mybir.dt.float32
    bf16 = mybir.dt.bfloat16
    mult = mybir.AluOpType.mult
    add = mybir.AluOpType.add

    with ExitStack() as ctx:
        tc = ctx.enter_context(tile.TileContext(nc))
        wpool = ctx.enter_context(tc.tile_pool(name="wpool", bufs=1))
        xpool = ctx.enter_context(tc.tile_pool(name="xpool", bufs=6))
        ypsum = ctx.enter_context(tc.tile_pool(name="ypsum", bufs=4, space="PSUM"))
        cpsum = ctx.enter_context(tc.tile_pool(name="cpsum", bufs=1, space="PSUM"))
        opsum = ctx.enter_context(tc.tile_pool(name="opsum", bufs=2, space="PSUM"))
        ypool = ctx.enter_context(tc.tile_pool(name="ypool", bufs=3))
        spool = ctx.enter_context(tc.tile_pool(name="spool", bufs=3))
        opool = ctx.enter_context(tc.tile_pool(name="opool", bufs=3))

        # --- persistent tiles ---
        wc_sb = wpool.tile([128, KC, 96], bf16)
        nc.sync.dma_start(out=wc_sb[:], in_=wc.ap().rearrange("c p o -> p c o"))
        wfc_sb = wpool.tile([P + 1, C], f32)
        nc.sync.dma_start(out=wfc_sb[:], in_=wfc.ap())
        ident = wpool.tile([128, 128], f32)
        make_identity(nc, ident[:])

        out_ap = out.ap()
        xt_ap = xt.ap()

        for g in range(ngroups):
            gs = g * GROUP
            # --- load x^T slabs: [128 (d-in-chunk), KC, SLAB] bf16 ---
            slabs = []
            for h in range(SPG):
                xs = xpool.tile([128, KC, SLAB], bf16)
                nc.sync.dma_start(out=xs[:], in_=xt_ap[g * SPG + h])
                slabs.append(xs)

            # --- projections: per 128-sample subtile, 12 matmuls -> y [128, 288]
            y_list = []
            for j in range(SUB):
                y_ps = ypsum.tile([128, 3 * 96], f32)
                xs = slabs[(j * 128) // SLAB]
                jj = (j * 128) % SLAB
                for c in range(KC):
                    t = c // 4
                    nc.tensor.matmul(
                        y_ps[:, 96 * t : 96 * t + 96],
                        xs[:, c, jj : jj + 128],
                        wc_sb[:, c, :],
                        start=(c % 4 == 0),
                        stop=(c % 4 == 3),
                    )
                y_list.append(y_ps)

            # --- copy y PSUM -> SBUF (bf16), split DVE/ACT ---
            Y = ypool.tile([128, SUB, 288], bf16)
            copy_insts = []
            with tc.high_priority(offset=GPRIO):
                for j, y_ps in enumerate(y_list):
                    copy_insts.append(nc.scalar.copy(out=Y[:, j, :], in_=y_ps[:, :]))

            # layout per subtile: [k0 v0 qp0 | k1 v1 qp1 | k2 v2 qp2] blocks of 96
            # (block t at 96*t: k at +0, v at +32, qp at +64)
            # --- q = qp0 + qp1 + qp2 (into the qp0 slot, cols 64:96) ---
            nc.vector.tensor_tensor(
                out=Y[:, :, 64:96], in0=Y[:, :, 64:96], in1=Y[:, :, 160:192], op=add
            )
            nc.vector.tensor_tensor(
                out=Y[:, :, 64:96], in0=Y[:, :, 64:96], in1=Y[:, :, 256:288], op=add
            )

            # --- logits_t = sum_p q*k_t ---
            m_scr = spool.tile([128, SUB, 3, 32], f32)
            q_b = _ins_dim(Y[:, :, 64:96], 2, 3, 0)        # [128, SUB, 3, 32], t bcast
            k_v = _ins_dim(Y[:, :, 0:32], 2, 3, 96)        # k_t at 96*t
            nc.vector.tensor_tensor(out=m_scr[:], in0=q_b, in1=k_v, op=mult)
            logits = spool.tile([128, SUB, 3], f32)
            nc.vector.tensor_reduce(
                out=logits[:], in_=m_scr[:], axis=mybir.AxisListType.X, op=add
            )

            # --- e = exp(logits), Z = sum_t e, R = 1/Z ---
            E = spool.tile([128, SUB, 3], f32)
            exp_inst = nc.scalar.activation(
                out=E[:], in_=logits[:], func=mybir.ActivationFunctionType.Exp
            )
            for ci in copy_insts:
                bass._add_dep_helper(exp_inst.ins, ci.ins, reason="keep ACT copies ahead of exp")
            Z = spool.tile([128, SUB, 1], f32)
            nc.vector.tensor_reduce(
                out=Z[:], in_=E[:], axis=mybir.AxisListType.X, op=add
            )
            R = spool.tile([128, SUB, 1], f32)
            nc.vector.reciprocal(out=R[:], in_=Z[:])

            # --- u = sum_t e_t * v_t ; U33 = [u | Z] ---
            s_scr = spool.tile([128, SUB, 32, 3], f32)
            v_v = _ins_dim(Y[:, :, 32:64], 3, 3, 96)       # dims (g, o, t)
            e_b = _ins_dim(E[:, :, :], 2, 32, 0)           # dims (g, o, t)
            nc.vector.tensor_tensor(out=s_scr[:], in0=v_v, in1=e_b, op=mult)
            U33 = spool.tile([128, SUB, P + 1], f32)
            nc.vector.tensor_reduce(
                out=U33[:, :, 0:32], in_=s_scr[:], axis=mybir.AxisListType.X, op=add
            )
            nc.vector.tensor_copy(out=U33[:, :, 32:33], in_=Z[:])

            # --- cT = U33^T via PE transpose; FC; scale by R ---
            # deferred by ~one group of priority so the PE queue never stalls
            # on the DVE epilogue chain between projection bursts
            ctx_prio = tc.high_priority(offset=-GPRIO)
            ctx_prio.__enter__()
            ct_ps = cpsum.tile([P + 1, GROUP], f32)
            for j in range(SUB):
                nc.tensor.transpose(
                    ct_ps[:, j * 128 : (j + 1) * 128], U33[:, j, :], ident[:]
                )
            ct_sb = spool.tile([P + 1, GROUP], f32)
            nc.scalar.copy(out=ct_sb[:], in_=ct_ps[:])

            o_ps = opsum.tile([128, SUB, C], f32)
            for j in range(SUB):
                nc.tensor.matmul(
                    o_ps[:, j, :],
                    ct_sb[:, j * 128 : (j + 1) * 128],
                    wfc_sb[:],
                    start=True,
                    stop=True,
                )
            out_sb = opool.tile([128, SUB, C], f32)
            r_b = _ins_dim(R[:, :, 0], 2, C, 0)            # [128, SUB, C], bcast over C
            nc.vector.tensor_tensor(out=out_sb[:], in0=o_ps[:], in1=r_b, op=mult)

            nc.scalar.dma_start(
                out=out_ap[gs : gs + GROUP, :].rearrange("(j p) c -> p j c", p=128),
                in_=out_sb[:],
            )
            ctx_prio.__exit__(None, None, None)

    nc.finalize()
    _split_excess_waits(nc)
    return nc


def _split_excess_waits(nc):
    """walrus rejects >1 sync wait on compute instruction structs; hoist the
    extras onto same-engine NoOps inserted just before the offender."""
    exempt = (mybir.InstEventSemaphore,)
    for func in nc.m.functions:
        for blk in func.blocks:
            insts = list(blk.instructions)
            out_list = []
            changed = False
            for inst in insts:
                si = getattr(inst, "sync_info", None)
                ow = list(si.on_wait) if (si is not None and si.on_wait) else []
                if len(ow) > 1 and not isinstance(inst, exempt):
                    for w in ow[:-1]:
                        nop = mybir.InstNoOp(
                            name=nc.get_next_instruction_name(),
                            engine=inst.engine,
                            sync_info=mybir.SyncInfo(on_wait=[w], on_update=[]),
                            bass_nofuse=True,
                        )
                        out_list.append(nop)
                    si.on_wait = [ow[-1]]
                    changed = True
                out_list.append(inst)
            if changed:
                blk.instructions = out_list


_NC_CACHE = {}


def _get_nc(nb):
    if nb not in _NC_CACHE:
        _NC_CACHE[nb] = build_nc(nb)
    return _NC_CACHE[nb]


def _prep_weights(Wk, Wv, Wq, Wfc, bfc):
    WkT = Wk.T.astype(np.float32)   # [512, 32]
    WvT = Wv.T.astype(np.float32)   # [512, 32]
    WqT = Wq.T.astype(np.float32)   # [1536, 32]
    wc = np.zeros((KC, 128, 96), np.float32)
    for c in range(KC):
        t, dsub = divmod(c, 4)
        d512 = slice(dsub * 128, (dsub + 1) * 128)
        rows = slice(c * 128, (c + 1) * 128)
        wc[c, :, 0:32] = WkT[d512]
        wc[c, :, 32:64] = WvT[d512]
        wc[c, :, 64:96] = WqT[rows]
    wc = wc.astype(BF16)
    wfc_aug = np.concatenate(
        [Wfc.T.astype(np.float32), bfc.reshape(1, C).astype(np.float32)], axis=0
    )  # [33, 10]
    return wc, wfc_aug


LAST_RESULT = None


def kernel(x, Wk, Wv, Wq, Wfc, bfc):
    global LAST_RESULT
    x = np.asarray(x, dtype=np.float32)
    Wk = np.asarray(Wk, dtype=np.float32)
    Wv = np.asarray(Wv, dtype=np.float32)
    Wq = np.asarray(Wq, dtype=np.float32)
    Wfc = np.asarray(Wfc, dtype=np.float32)
    bfc = np.asarray(bfc, dtype=np.float32)

    B = x.shape[0]
    assert B % NCORES == 0
    nb = B // NCORES
    nc = _get_nc(nb)
    wc, wfc_aug = _prep_weights(Wk, Wv, Wq, Wfc, bfc)

    xr = x.reshape(NCORES, nb, DF)
    in_maps = []
    for i in range(NCORES):
        # xt[h, p, c, s] = xT[c*128+p, h*SLAB+s]
        xt = np.ascontiguousarray(
            xr[i]
            .astype(BF16)
            .T.reshape(KC, 128, nb // SLAB, SLAB)
            .transpose(2, 1, 0, 3)
        )
        in_maps.append({"xt": xt, "wc": wc, "wfc": wfc_aug})

    LAST_RESULT = run_bass_kernel_spmd(nc, in_maps, core_ids=list(range(NCORES)))
    res = LAST_RESULT.results
    out = np.concatenate([res[i]["out"] for i in range(NCORES)], axis=0)
    return out.astype(np.float32)
